# revision 18
# baseline (speedup 1.0000x reference)
"""Trainium2 Bass kernel: grouped similarity-gating normalization.

Reference computation (per batch b, group g, cpg=64 channels, hw=784):
    means[c]  = mean_hw(x[c, :])
    s[hw]     = sum_c x[c, hw] * means[c]
    t         = (s - mean(s)) * rsqrt(var(s) + eps)
    gate      = sigmoid(t * weight[g] + bias[g])
    out[c,hw] = x[c, hw] * gate[hw]

Sharding: data-parallel over batch B=64 across 8 cores (8 batches/core).

Per-core layout: one SBUF tile [128, 4, 784] per batch holds channels
c = 4*p + j (p = partition, j = free chunk) -> contiguous 1.6MB DMAs, and
group(c) = c//64 = p//16, i.e. each group owns a 16-partition band.

  - channel means via one DVE reduce (innermost axis of [128,4,784])
  - s (contraction over channels) via PE: 4 accumulating matmuls with
    lhsT[p, g] = means[p,j] masked to group bands (constant 0/1 indicator
    m8 times means). An extra N=1 matmul column with rhs=means gives
    mu = mean(s) = sum_c means[c]^2 for free.
  - stats on ScalarE: Square+accum_out -> sum(s^2); gate computed as
    sigmoid(s*a + c) in one activation with per-partition scale/bias APs,
    where a = rstd*weight[g], c = bias[g] - mu*a.
  - gate broadcast to the 128 partitions via PE with the transposed
    indicator (mt), then one DVE multiply (j-broadcast AP), DMA out.
"""

import sys

if "/opt/trn_rl_repo" not in sys.path:
    sys.path.insert(0, "/opt/trn_rl_repo")

from contextlib import ExitStack

import numpy as np

import concourse.bacc as bacc
import concourse.bass as bass
import concourse.tile as tile
from concourse import mybir
from concourse.bass_utils import run_bass_kernel_spmd

B, C, H, W = 64, 512, 28, 28
G = 8
HW = H * W          # 784
NCORES = 8
BLOC = B // NCORES  # 8 batches per core
NP = 128            # SBUF partitions
NJ = C // NP        # 4 channel chunks per partition (c = NJ*p + j)
PBAND = NP // G     # 16 partitions per group
EPS = 1e-5
F32 = mybir.dt.float32
MMCHUNK = 512       # max fp32 moving free dim per matmul

_cache: dict = {}

# implementation choices (bisectable)
BCAST_MODE = "dma"   # "dma" (SBUF->SBUF SWDGE) or "pe" (indicator matmul)
OUT_ENGINE = "scalar"  # "scalar" or "sync" HWDGE ring for output DMAs
MUL_J3 = "gpsimd"    # "gpsimd" or "vector" for the last gating multiply
REDUCE_MODE = "split"  # "split" (DVE j01 + ACT copy-accum j23) or "dve" (one reduce)
# NOTE: tensor_tensor_reduce (custom DVE ucode op) wedges the device under
# the axon/PJRT runtime (NRT_EXEC_UNIT_UNRECOVERABLE) -- keep "plain".
SQ2_MODE = "plain"   # "ttr" (tensor_tensor_reduce) or "plain" (mul + reduce)


def _emit(tc, nc, xs, m8, mt, wv, bv, ys):
    AF = mybir.ActivationFunctionType
    OP = mybir.AluOpType
    PREF = 3  # input prefetch depth (batches)
    with ExitStack() as ctx:
        consts = ctx.enter_context(tc.tile_pool(name="consts", bufs=1))
        xpool = ctx.enter_context(tc.tile_pool(name="xpool", bufs=BLOC))
        mpool = ctx.enter_context(tc.tile_pool(name="mpool", bufs=3))
        vpool = ctx.enter_context(tc.tile_pool(name="vpool", bufs=4))
        gpool = ctx.enter_context(tc.tile_pool(name="gpool", bufs=3))
        spsum = ctx.enter_context(tc.tile_pool(name="spsum", bufs=2, space="PSUM"))

        m8_sb = consts.tile([NP, G], F32)
        nc.sync.dma_start(out=m8_sb[:], in_=m8[:])
        mt_sb = consts.tile([G, NP], F32)
        nc.sync.dma_start(out=mt_sb[:], in_=mt[:])
        wv_sb = consts.tile([G, 1], F32)
        nc.sync.dma_start(out=wv_sb[:], in_=wv[:])
        bv_sb = consts.tile([G, 1], F32)
        nc.sync.dma_start(out=bv_sb[:], in_=bv[:])
        eps_sb = consts.tile([G, 1], F32)
        nc.vector.memset(eps_sb[:], EPS)

        xts = {}

        def dma_in(b):
            xt = xpool.tile([NP, NJ, HW], F32)
            # two chunks so the means reduce can start on the first half
            nc.sync.dma_start(out=xt[:, 0:2, :], in_=xs[b, :, 0:2, :])
            nc.sync.dma_start(out=xt[:, 2:4, :], in_=xs[b, :, 2:4, :])
            xts[b] = xt

        for b in range(PREF):
            dma_in(b)

        for b in range(BLOC):
            xt = xts.pop(b)

            # per-channel spatial means: j=0,1 on DVE, j=2,3 on ScalarE
            means = mpool.tile([NP, NJ], F32, tag="means")
            if REDUCE_MODE == "split":
                sums01 = mpool.tile([NP, 2], F32, tag="sums01")
                nc.vector.reduce_sum(out=sums01[:], in_=xt[:, 0:2, :], axis=mybir.AxisListType.X)
                nc.vector.tensor_scalar_mul(means[:, 0:2], sums01[:], 1.0 / HW)
                cps = gpool.tile([NP, HW], F32, tag="cps")
                for j in (2, 3):
                    nc.scalar.activation(
                        out=cps[:], in_=xt[:, j, :], func=AF.Copy,
                        scale=1.0 / HW, accum_out=means[:, j : j + 1],
                    )
            else:
                sums = mpool.tile([NP, NJ], F32, tag="sums")
                nc.vector.reduce_sum(out=sums[:], in_=xt[:], axis=mybir.AxisListType.X)
                nc.vector.tensor_scalar_mul(means[:], sums[:], 1.0 / HW)

            # lhsT[:, j, g] = means[p, j] masked to group band g
            lhsT = mpool.tile([NP, NJ, G], F32, tag="lhsT")
            for j in range(NJ):
                nc.vector.tensor_scalar_mul(lhsT[:, j, :], m8_sb[:], means[:, j : j + 1])

            # sq2[p] = sum_j means[p,j]^2  (for mu[g] = sum_{p in g} sq2[p])
            msq = mpool.tile([NP, NJ], F32, tag="msq")
            sq2 = mpool.tile([NP, 1], F32, tag="sq2")
            if SQ2_MODE == "ttr":
                nc.vector.tensor_tensor_reduce(
                    out=msq[:], in0=means[:], in1=means[:], scale=1.0, scalar=0.0,
                    op0=OP.mult, op1=OP.add, accum_out=sq2[:],
                )
            else:
                nc.vector.tensor_mul(msq[:], means[:], means[:])
                nc.vector.reduce_sum(out=sq2[:], in_=msq[:], axis=mybir.AxisListType.X)

            # s[g, hw] (cols 0:HW) via accumulating matmuls; mu[g] in col HW
            ps = spsum.tile([G, HW + 1], F32)
            for c0 in range(0, HW, MMCHUNK):
                c1 = min(c0 + MMCHUNK, HW)
                for j in range(NJ):
                    st = dict(start=(j == 0), stop=(j == NJ - 1))
                    nc.tensor.matmul(ps[:, c0:c1], lhsT[:, j, :], xt[:, j, c0:c1], **st)
            nc.tensor.matmul(ps[:, HW : HW + 1], m8_sb[:], sq2[:])

            # stats: nmu = -mu; hwvar = sum((s-mu)^2); std = sqrt(var+eps)
            nmu = vpool.tile([G, 1], F32, tag="nmu")
            nc.vector.tensor_scalar_mul(nmu[:], ps[:, HW : HW + 1], -1.0)
            sq = gpool.tile([G, HW], F32, tag="sq")
            hwvar = vpool.tile([G, 1], F32, tag="hwvar")
            nc.scalar.activation(
                out=sq[:], in_=ps[:, 0:HW], func=AF.Square, bias=nmu[:], accum_out=hwvar[:]
            )
            std = vpool.tile([G, 1], F32, tag="std")
            nc.scalar.activation(
                out=std[:], in_=hwvar[:], func=AF.Sqrt, scale=1.0 / HW, bias=eps_sb[:]
            )
            rstd = vpool.tile([G, 1], F32, tag="rstd")
            nc.vector.reciprocal(rstd[:], std[:])
            # gate = sigmoid(s*a + c), a = rstd*w, c = bias + nmu*a
            a_t = vpool.tile([G, 1], F32, tag="a_t")
            nc.vector.tensor_mul(a_t[:], rstd[:], wv_sb[:])
            c_t = vpool.tile([G, 1], F32, tag="c_t")
            nc.vector.scalar_tensor_tensor(
                out=c_t[:], in0=nmu[:], scalar=a_t[:], in1=bv_sb[:],
                op0=OP.mult, op1=OP.add,
            )
            gate = gpool.tile([G, HW], F32, tag="gate")
            nc.scalar.activation(
                out=gate[:], in_=ps[:, 0:HW], func=AF.Sigmoid, bias=c_t[:], scale=a_t[:]
            )

            # broadcast gate rows to 16-partition bands
            if BCAST_MODE == "dma":
                bg = gpool.tile([NP, HW], F32, tag="bg")
                gate_ap = gate[:]
                bcast_src = bass.AP(
                    tensor=gate_ap.tensor,
                    offset=gate_ap.offset,
                    ap=[list(gate_ap.ap[0]), [0, PBAND], list(gate_ap.ap[1])],
                )
                nc.gpsimd.dma_start(out=bg[:], in_=bcast_src)
                bg_ap = bg[:]
            else:
                bgp = spsum.tile([NP, HW], F32, tag="bgp")
                for c0 in range(0, HW, MMCHUNK):
                    c1 = min(c0 + MMCHUNK, HW)
                    nc.tensor.matmul(bgp[:, c0:c1], mt_sb[:], gate[:, c0:c1])
                bg_ap = bgp[:]

            # out = x * gate, in place; j=0..2 on DVE, j=3 on GpSimd
            for j in range(NJ - 1):
                nc.vector.tensor_mul(xt[:, j, :], xt[:, j, :], bg_ap)
            if MUL_J3 == "gpsimd":
                nc.gpsimd.tensor_mul(xt[:, NJ - 1, :], xt[:, NJ - 1, :], bg_ap)
            else:
                nc.vector.tensor_mul(xt[:, NJ - 1, :], xt[:, NJ - 1, :], bg_ap)

            # store (one contiguous DMA per batch)
            if OUT_ENGINE == "scalar":
                nc.scalar.dma_start(out=ys[b], in_=xt[:])
            else:
                nc.sync.dma_start(out=ys[b], in_=xt[:])

            if b + PREF < BLOC:
                dma_in(b + PREF)


def _build_nc():
    nc = bacc.Bacc("TRN2", debug=False)
    xs = nc.dram_tensor("xs", [BLOC, NP, NJ, HW], F32, kind="ExternalInput")
    m8 = nc.dram_tensor("m8", [NP, G], F32, kind="ExternalInput")
    mt = nc.dram_tensor("mt", [G, NP], F32, kind="ExternalInput")
    wv = nc.dram_tensor("wv", [G, 1], F32, kind="ExternalInput")
    bv = nc.dram_tensor("bv", [G, 1], F32, kind="ExternalInput")
    ys = nc.dram_tensor("ys", [BLOC, NP, NJ, HW], F32, kind="ExternalOutput")
    with tile.TileContext(nc) as tc:
        _emit(tc, nc, xs, m8, mt, wv, bv, ys)
    nc.compile()
    return nc


def get_nc():
    if "nc" not in _cache:
        _cache["nc"] = _build_nc()
    return _cache["nc"]


def make_in_maps(x, weight, bias):
    x = np.ascontiguousarray(np.asarray(x, dtype=np.float32))
    weight = np.asarray(weight, dtype=np.float32).reshape(G)
    bias = np.asarray(bias, dtype=np.float32).reshape(G)
    # [core, b, p, j, hw] with c = NJ*p + j
    xs = x.reshape(NCORES, BLOC, NP, NJ, HW)
    m8 = np.zeros((NP, G), dtype=np.float32)
    m8[np.arange(NP), np.arange(NP) // PBAND] = 1.0
    mt = np.ascontiguousarray(m8.T)
    wv = np.ascontiguousarray(weight[:, None])
    bv = np.ascontiguousarray(bias[:, None])
    return [
        {"xs": np.ascontiguousarray(xs[i]), "m8": m8, "mt": mt, "wv": wv, "bv": bv}
        for i in range(NCORES)
    ]


def run(x, weight, bias, trace=False, **spmd_kwargs):
    nc = get_nc()
    in_maps = make_in_maps(x, weight, bias)
    res = run_bass_kernel_spmd(
        nc, in_maps, core_ids=list(range(NCORES)), trace=trace, **spmd_kwargs
    )
    out = np.stack([res.results[i]["ys"] for i in range(NCORES)])
    return out.reshape(B, C, H, W), res


def kernel(x, weight, bias, groups=G, **_ignored):
    assert int(groups) == G
    out, _ = run(x, weight, bias, trace=False)
    return out


# revision 20
# speedup vs baseline: 1.1199x; 1.1199x over previous
"""Trainium2 Bass kernel: grouped similarity-gating normalization.

Reference computation (per batch b, group g, cpg=64 channels, hw=784):
    means[c]  = mean_hw(x[c, :])
    s[hw]     = sum_c x[c, hw] * means[c]
    t         = (s - mean(s)) * rsqrt(var(s) + eps)
    gate      = sigmoid(t * weight[g] + bias[g])
    out[c,hw] = x[c, hw] * gate[hw]

Sharding: data-parallel over batch B=64 across 8 cores (8 batches/core).

Per-core layout: one SBUF tile [128, 4, 784] per batch holds channels
c = 4*p + j (p = partition, j = free chunk) -> contiguous 1.6MB DMAs, and
group(c) = c//64 = p//16, i.e. each group owns a 16-partition band.

  - channel means via one DVE reduce (innermost axis of [128,4,784])
  - s (contraction over channels) via PE: 4 accumulating matmuls with
    lhsT[p, g] = means[p,j] masked to group bands (constant 0/1 indicator
    m8 times means). An extra N=1 matmul column with rhs=means gives
    mu = mean(s) = sum_c means[c]^2 for free.
  - stats on ScalarE: Square+accum_out -> sum(s^2); gate computed as
    sigmoid(s*a + c) in one activation with per-partition scale/bias APs,
    where a = rstd*weight[g], c = bias[g] - mu*a.
  - gate broadcast to the 128 partitions via PE with the transposed
    indicator (mt), then one DVE multiply (j-broadcast AP), DMA out.
"""

import sys

if "/opt/trn_rl_repo" not in sys.path:
    sys.path.insert(0, "/opt/trn_rl_repo")

from contextlib import ExitStack

import numpy as np

import concourse.bacc as bacc
import concourse.bass as bass
import concourse.tile as tile
from concourse import mybir
from concourse.bass_utils import run_bass_kernel_spmd

B, C, H, W = 64, 512, 28, 28
G = 8
HW = H * W          # 784
NCORES = 8
BLOC = B // NCORES  # 8 batches per core
NP = 128            # SBUF partitions
NJ = C // NP        # 4 channel chunks per partition (c = NJ*p + j)
PBAND = NP // G     # 16 partitions per group
EPS = 1e-5
F32 = mybir.dt.float32
MMCHUNK = 512       # max fp32 moving free dim per matmul

_cache: dict = {}

# implementation choices (bisectable)
BCAST_MODE = "dma"   # "dma" (SBUF->SBUF SWDGE) or "pe" (indicator matmul)
OUT_ENGINE = "sync"  # "scalar" or "sync" HWDGE ring for output DMAs
MUL_J3 = "vector"    # "gpsimd" or "vector" for the last gating multiply
REDUCE_MODE = "split"  # "split" (DVE j01 + ACT copy-accum j23) or "dve" (one reduce)
# NOTE: tensor_tensor_reduce (custom DVE ucode op) wedges the device under
# the axon/PJRT runtime (NRT_EXEC_UNIT_UNRECOVERABLE) -- keep "plain".
SQ2_MODE = "plain"   # "ttr" (tensor_tensor_reduce) or "plain" (mul + reduce)


def _emit(tc, nc, xs, m8, mt, wv, bv, ys):
    AF = mybir.ActivationFunctionType
    OP = mybir.AluOpType
    PREF = 3  # input prefetch depth (batches)
    with ExitStack() as ctx:
        consts = ctx.enter_context(tc.tile_pool(name="consts", bufs=1))
        xpool = ctx.enter_context(tc.tile_pool(name="xpool", bufs=BLOC))
        mpool = ctx.enter_context(tc.tile_pool(name="mpool", bufs=3))
        vpool = ctx.enter_context(tc.tile_pool(name="vpool", bufs=4))
        gpool = ctx.enter_context(tc.tile_pool(name="gpool", bufs=3))
        spsum = ctx.enter_context(tc.tile_pool(name="spsum", bufs=2, space="PSUM"))

        m8_sb = consts.tile([NP, G], F32)
        nc.sync.dma_start(out=m8_sb[:], in_=m8[:])
        mt_sb = consts.tile([G, NP], F32)
        nc.sync.dma_start(out=mt_sb[:], in_=mt[:])
        wv_sb = consts.tile([G, 1], F32)
        nc.sync.dma_start(out=wv_sb[:], in_=wv[:])
        bv_sb = consts.tile([G, 1], F32)
        nc.sync.dma_start(out=bv_sb[:], in_=bv[:])
        eps_sb = consts.tile([G, 1], F32)
        nc.vector.memset(eps_sb[:], EPS)

        xts = {}
        state = {}

        def dma_in(b):
            xt = xpool.tile([NP, NJ, HW], F32)
            # two chunks so the means reduce can start on the first half
            nc.sync.dma_start(out=xt[:, 0:2, :], in_=xs[b, :, 0:2, :])
            nc.sync.dma_start(out=xt[:, 2:4, :], in_=xs[b, :, 2:4, :])
            xts[b] = xt

        def phase1(b):
            # means + masked lhsT + sq2 (all pre-matmul per-batch prep)
            xt = xts[b]
            means = mpool.tile([NP, NJ], F32, tag="means")
            if REDUCE_MODE == "split":
                sums01 = mpool.tile([NP, 2], F32, tag="sums01")
                nc.vector.reduce_sum(out=sums01[:], in_=xt[:, 0:2, :], axis=mybir.AxisListType.X)
                nc.vector.tensor_scalar_mul(means[:, 0:2], sums01[:], 1.0 / HW)
                cps = gpool.tile([NP, HW], F32, tag="cps")
                for j in (2, 3):
                    nc.scalar.activation(
                        out=cps[:], in_=xt[:, j, :], func=AF.Copy,
                        scale=1.0 / HW, accum_out=means[:, j : j + 1],
                    )
            else:
                sums = mpool.tile([NP, NJ], F32, tag="sums")
                nc.vector.reduce_sum(out=sums[:], in_=xt[:], axis=mybir.AxisListType.X)
                nc.vector.tensor_scalar_mul(means[:], sums[:], 1.0 / HW)

            lhsT = mpool.tile([NP, NJ, G], F32, tag="lhsT")
            for j in range(NJ):
                nc.vector.tensor_scalar_mul(lhsT[:, j, :], m8_sb[:], means[:, j : j + 1])

            msq = mpool.tile([NP, NJ], F32, tag="msq")
            sq2 = mpool.tile([NP, 1], F32, tag="sq2")
            if SQ2_MODE == "ttr":
                nc.vector.tensor_tensor_reduce(
                    out=msq[:], in0=means[:], in1=means[:], scale=1.0, scalar=0.0,
                    op0=OP.mult, op1=OP.add, accum_out=sq2[:],
                )
            else:
                nc.vector.tensor_mul(msq[:], means[:], means[:])
                nc.vector.reduce_sum(out=sq2[:], in_=msq[:], axis=mybir.AxisListType.X)
            state[b] = (lhsT, sq2)

        def phase2(b):
            # s[g, hw] (cols 0:HW) via accumulating matmuls; mu[g] in col HW
            xt = xts[b]
            lhsT, sq2 = state[b]
            ps = spsum.tile([G, HW + 1], F32)
            for c0 in range(0, HW, MMCHUNK):
                c1 = min(c0 + MMCHUNK, HW)
                for j in range(NJ):
                    st = dict(start=(j == 0), stop=(j == NJ - 1))
                    nc.tensor.matmul(ps[:, c0:c1], lhsT[:, j, :], xt[:, j, c0:c1], **st)
            nc.tensor.matmul(ps[:, HW : HW + 1], m8_sb[:], sq2[:])
            state[b] = ps

        def phase3(b):
            # stats + gate + broadcast
            ps = state[b]
            nmu = vpool.tile([G, 1], F32, tag="nmu")
            nc.vector.tensor_scalar_mul(nmu[:], ps[:, HW : HW + 1], -1.0)
            sq = gpool.tile([G, HW], F32, tag="sq")
            hwvar = vpool.tile([G, 1], F32, tag="hwvar")
            nc.scalar.activation(
                out=sq[:], in_=ps[:, 0:HW], func=AF.Square, bias=nmu[:], accum_out=hwvar[:]
            )
            std = vpool.tile([G, 1], F32, tag="std")
            nc.scalar.activation(
                out=std[:], in_=hwvar[:], func=AF.Sqrt, scale=1.0 / HW, bias=eps_sb[:]
            )
            rstd = vpool.tile([G, 1], F32, tag="rstd")
            nc.vector.reciprocal(rstd[:], std[:])
            a_t = vpool.tile([G, 1], F32, tag="a_t")
            nc.vector.tensor_mul(a_t[:], rstd[:], wv_sb[:])
            c_t = vpool.tile([G, 1], F32, tag="c_t")
            nc.vector.scalar_tensor_tensor(
                out=c_t[:], in0=nmu[:], scalar=a_t[:], in1=bv_sb[:],
                op0=OP.mult, op1=OP.add,
            )
            gate = gpool.tile([G, HW], F32, tag="gate")
            nc.scalar.activation(
                out=gate[:], in_=ps[:, 0:HW], func=AF.Sigmoid, bias=c_t[:], scale=a_t[:]
            )

            if BCAST_MODE == "dma":
                bg = gpool.tile([NP, HW], F32, tag="bg")
                gate_ap = gate[:]
                bcast_src = bass.AP(
                    tensor=gate_ap.tensor,
                    offset=gate_ap.offset,
                    ap=[list(gate_ap.ap[0]), [0, PBAND], list(gate_ap.ap[1])],
                )
                nc.gpsimd.dma_start(out=bg[:], in_=bcast_src)
                bg_ap = bg[:]
            else:
                bgp = spsum.tile([NP, HW], F32, tag="bgp")
                for c0 in range(0, HW, MMCHUNK):
                    c1 = min(c0 + MMCHUNK, HW)
                    nc.tensor.matmul(bgp[:, c0:c1], mt_sb[:], gate[:, c0:c1])
                bg_ap = bgp[:]
            state[b] = bg_ap

        def phase4(b):
            # gating multiply (in place) + store
            xt = xts.pop(b)
            bg_ap = state.pop(b)
            for j in range(NJ - 1):
                nc.vector.tensor_mul(xt[:, j, :], xt[:, j, :], bg_ap)
            if MUL_J3 == "gpsimd":
                nc.gpsimd.tensor_mul(xt[:, NJ - 1, :], xt[:, NJ - 1, :], bg_ap)
            else:
                nc.vector.tensor_mul(xt[:, NJ - 1, :], xt[:, NJ - 1, :], bg_ap)
            if OUT_ENGINE == "scalar":
                nc.scalar.dma_start(out=ys[b], in_=xt[:])
            else:
                nc.sync.dma_start(out=ys[b], in_=xt[:])
            if b + PREF < BLOC:
                dma_in(b + PREF)

        # software-pipelined emission: each engine's stream sees work in
        # data-readiness order, so in-order engines never head-of-line block
        for b in range(PREF):
            dma_in(b)
        phase1(0)
        phase2(0)
        for b in range(BLOC):
            if b + 1 < BLOC:
                phase1(b + 1)
            phase3(b)
            if b + 1 < BLOC:
                phase2(b + 1)
            phase4(b)


def _build_nc():
    nc = bacc.Bacc("TRN2", debug=False)
    xs = nc.dram_tensor("xs", [BLOC, NP, NJ, HW], F32, kind="ExternalInput")
    m8 = nc.dram_tensor("m8", [NP, G], F32, kind="ExternalInput")
    mt = nc.dram_tensor("mt", [G, NP], F32, kind="ExternalInput")
    wv = nc.dram_tensor("wv", [G, 1], F32, kind="ExternalInput")
    bv = nc.dram_tensor("bv", [G, 1], F32, kind="ExternalInput")
    ys = nc.dram_tensor("ys", [BLOC, NP, NJ, HW], F32, kind="ExternalOutput")
    with tile.TileContext(nc) as tc:
        _emit(tc, nc, xs, m8, mt, wv, bv, ys)
    nc.compile()
    return nc


def get_nc():
    if "nc" not in _cache:
        _cache["nc"] = _build_nc()
    return _cache["nc"]


def make_in_maps(x, weight, bias):
    x = np.ascontiguousarray(np.asarray(x, dtype=np.float32))
    weight = np.asarray(weight, dtype=np.float32).reshape(G)
    bias = np.asarray(bias, dtype=np.float32).reshape(G)
    # [core, b, p, j, hw] with c = NJ*p + j
    xs = x.reshape(NCORES, BLOC, NP, NJ, HW)
    m8 = np.zeros((NP, G), dtype=np.float32)
    m8[np.arange(NP), np.arange(NP) // PBAND] = 1.0
    mt = np.ascontiguousarray(m8.T)
    wv = np.ascontiguousarray(weight[:, None])
    bv = np.ascontiguousarray(bias[:, None])
    return [
        {"xs": np.ascontiguousarray(xs[i]), "m8": m8, "mt": mt, "wv": wv, "bv": bv}
        for i in range(NCORES)
    ]


def run(x, weight, bias, trace=False, **spmd_kwargs):
    nc = get_nc()
    in_maps = make_in_maps(x, weight, bias)
    res = run_bass_kernel_spmd(
        nc, in_maps, core_ids=list(range(NCORES)), trace=trace, **spmd_kwargs
    )
    out = np.stack([res.results[i]["ys"] for i in range(NCORES)])
    return out.reshape(B, C, H, W), res


def kernel(x, weight, bias, groups=G, **_ignored):
    assert int(groups) == G
    out, _ = run(x, weight, bias, trace=False)
    return out


# revision 28
# speedup vs baseline: 1.5591x; 1.3921x over previous
"""Trainium2 Bass kernel: grouped similarity-gating normalization.

Reference computation (per batch b, group g, cpg=64 channels, hw=784):
    means[c]  = mean_hw(x[c, :])
    s[hw]     = sum_c x[c, hw] * means[c]
    t         = (s - mean(s)) * rsqrt(var(s) + eps)
    gate      = sigmoid(t * weight[g] + bias[g])
    out[c,hw] = x[c, hw] * gate[hw]

Sharding: data-parallel over batch B=64 across 8 cores (8 batches/core).

Per-core layout: one SBUF tile [128, 4, 784] per batch holds channels
c = 4*p + j (p = partition, j = free chunk) -> contiguous 1.6MB DMAs, and
group(c) = c//64 = p//16, i.e. each group owns a 16-partition band.

  - channel means via one DVE reduce (innermost axis of [128,4,784])
  - s (contraction over channels) via PE: 4 accumulating matmuls with
    lhsT[p, g] = means[p,j] masked to group bands (constant 0/1 indicator
    m8 times means). An extra N=1 matmul column with rhs=means gives
    mu = mean(s) = sum_c means[c]^2 for free.
  - stats on ScalarE: Square+accum_out -> sum(s^2); gate computed as
    sigmoid(s*a + c) in one activation with per-partition scale/bias APs,
    where a = rstd*weight[g], c = bias[g] - mu*a.
  - gate broadcast to the 128 partitions via PE with the transposed
    indicator (mt), then one DVE multiply (j-broadcast AP), DMA out.
"""

import sys

if "/opt/trn_rl_repo" not in sys.path:
    sys.path.insert(0, "/opt/trn_rl_repo")

from contextlib import ExitStack

import numpy as np

import concourse.bacc as bacc
import concourse.bass as bass
import concourse.tile as tile
from concourse import mybir
from concourse.bass_utils import run_bass_kernel_spmd

B, C, H, W = 64, 512, 28, 28
G = 8
HW = H * W          # 784
NCORES = 8
BLOC = B // NCORES  # 8 batches per core
NP = 128            # SBUF partitions
NJ = C // NP        # 4 channel chunks per partition (c = NJ*p + j)
PBAND = NP // G     # 16 partitions per group
EPS = 1e-5
F32 = mybir.dt.float32
MMCHUNK = 512       # max fp32 moving free dim per matmul

_cache: dict = {}

# implementation choices (bisectable)
OUT_ENGINE = "sync"  # "scalar" or "sync" HWDGE ring for output DMAs
MUL_J3 = "gpsimd"    # "gpsimd" or "vector" for the last gating multiply
REDUCE_MODE = "split"  # "split" (DVE j01 + ACT copy-accum j23) or "dve" (one reduce)
# NOTE: tensor_tensor_reduce (custom DVE ucode op) wedges the device under
# the axon/PJRT runtime (NRT_EXEC_UNIT_UNRECOVERABLE) -- keep "plain".
SQ2_MODE = "plain"   # "ttr" (tensor_tensor_reduce) or "plain" (mul + reduce)


def _emit(tc, nc, xs, m8, wv, bv, ys):
    AF = mybir.ActivationFunctionType
    OP = mybir.AluOpType
    PREF = 3  # input prefetch depth (batches)
    with ExitStack() as ctx:
        consts = ctx.enter_context(tc.tile_pool(name="consts", bufs=1))
        xpool = ctx.enter_context(tc.tile_pool(name="xpool", bufs=BLOC))
        mpool = ctx.enter_context(tc.tile_pool(name="mpool", bufs=3))
        vpool = ctx.enter_context(tc.tile_pool(name="vpool", bufs=4))
        gpool = ctx.enter_context(tc.tile_pool(name="gpool", bufs=3))
        spsum = ctx.enter_context(tc.tile_pool(name="spsum", bufs=2, space="PSUM"))

        # m8 input now carries the [NP, NP] block-banded 0/1 indicator
        # M16[p, q] = (p//PBAND == q//PBAND); wv/bv are 16x-replicated [NP, 1]
        m16_sb = consts.tile([NP, NP], F32)
        nc.sync.dma_start(out=m16_sb[:], in_=m8[:])
        wv_sb = consts.tile([NP, 1], F32)
        nc.sync.dma_start(out=wv_sb[:], in_=wv[:])
        bv_sb = consts.tile([NP, 1], F32)
        nc.sync.dma_start(out=bv_sb[:], in_=bv[:])
        eps_sb = consts.tile([NP, 1], F32)
        nc.vector.memset(eps_sb[:], EPS)

        xts = {}
        state = {}

        def dma_in(b):
            xt = xpool.tile([NP, NJ, HW], F32)
            # two chunks so the means reduce can start on the first half
            nc.sync.dma_start(out=xt[:, 0:2, :], in_=xs[b, :, 0:2, :])
            nc.sync.dma_start(out=xt[:, 2:4, :], in_=xs[b, :, 2:4, :])
            xts[b] = xt

        def phase1(b):
            # means + masked lhsT + sq2 (all pre-matmul per-batch prep)
            xt = xts[b]
            means = mpool.tile([NP, NJ], F32, tag="means")
            if REDUCE_MODE == "split":
                sums01 = mpool.tile([NP, 2], F32, tag="sums01")
                nc.vector.reduce_sum(out=sums01[:], in_=xt[:, 0:2, :], axis=mybir.AxisListType.X)
                nc.vector.tensor_scalar_mul(means[:, 0:2], sums01[:], 1.0 / HW)
                cps = gpool.tile([NP, HW], F32, tag="cps")
                for j in (2, 3):
                    nc.scalar.activation(
                        out=cps[:], in_=xt[:, j, :], func=AF.Copy,
                        scale=1.0 / HW, accum_out=means[:, j : j + 1],
                    )
            else:
                sums = mpool.tile([NP, NJ], F32, tag="sums")
                nc.vector.reduce_sum(out=sums[:], in_=xt[:], axis=mybir.AxisListType.X)
                nc.vector.tensor_scalar_mul(means[:], sums[:], 1.0 / HW)

            # lhsT[:, j, q] = means[p, j] masked to the 16-wide band of q, so the
            # matmul emits s replicated onto all 128 PSUM partitions (M=128 is
            # free: PE cost is N-bound)
            lhsT = mpool.tile([NP, NJ, NP], F32, tag="lhsT")
            for j in range(NJ):
                nc.vector.tensor_scalar_mul(lhsT[:, j, :], m16_sb[:], means[:, j : j + 1])

            msq = mpool.tile([NP, NJ], F32, tag="msq")
            sq2 = mpool.tile([NP, 1], F32, tag="sq2")
            if SQ2_MODE == "ttr":
                nc.vector.tensor_tensor_reduce(
                    out=msq[:], in0=means[:], in1=means[:], scale=1.0, scalar=0.0,
                    op0=OP.mult, op1=OP.add, accum_out=sq2[:],
                )
            else:
                nc.vector.tensor_mul(msq[:], means[:], means[:])
                nc.vector.reduce_sum(out=sq2[:], in_=msq[:], axis=mybir.AxisListType.X)
            state[b] = (lhsT, sq2)

        def phase2(b):
            # s (replicated per 16-band) in cols 0:HW; replicated mu in col HW
            xt = xts[b]
            lhsT, sq2 = state[b]
            ps = spsum.tile([NP, HW + 1], F32)
            for c0 in range(0, HW, MMCHUNK):
                c1 = min(c0 + MMCHUNK, HW)
                for j in range(NJ):
                    st = dict(start=(j == 0), stop=(j == NJ - 1))
                    nc.tensor.matmul(ps[:, c0:c1], lhsT[:, j, :], xt[:, j, c0:c1], **st)
            nc.tensor.matmul(ps[:, HW : HW + 1], m16_sb[:], sq2[:])
            state[b] = ps

        def phase3(b):
            # stats + gate (everything already replicated on 128 partitions)
            ps = state[b]
            nmu = vpool.tile([NP, 1], F32, tag="nmu")
            nc.vector.tensor_scalar_mul(nmu[:], ps[:, HW : HW + 1], -1.0)
            sq = gpool.tile([NP, HW], F32, tag="sq")
            hwvar = vpool.tile([NP, 1], F32, tag="hwvar")
            nc.scalar.activation(
                out=sq[:], in_=ps[:, 0:HW], func=AF.Square, bias=nmu[:], accum_out=hwvar[:]
            )
            std = vpool.tile([NP, 1], F32, tag="std")
            nc.scalar.activation(
                out=std[:], in_=hwvar[:], func=AF.Sqrt, scale=1.0 / HW, bias=eps_sb[:]
            )
            rstd = vpool.tile([NP, 1], F32, tag="rstd")
            nc.vector.reciprocal(rstd[:], std[:])
            a_t = vpool.tile([NP, 1], F32, tag="a_t")
            nc.vector.tensor_mul(a_t[:], rstd[:], wv_sb[:])
            c_t = vpool.tile([NP, 1], F32, tag="c_t")
            nc.vector.scalar_tensor_tensor(
                out=c_t[:], in0=nmu[:], scalar=a_t[:], in1=bv_sb[:],
                op0=OP.mult, op1=OP.add,
            )
            gate = gpool.tile([NP, HW], F32, tag="gate")
            nc.scalar.activation(
                out=gate[:], in_=ps[:, 0:HW], func=AF.Sigmoid, bias=c_t[:], scale=a_t[:]
            )
            state[b] = gate[:]

        def phase4(b):
            # gating multiply (in place) + store
            xt = xts.pop(b)
            bg_ap = state.pop(b)
            for j in range(NJ - 1):
                nc.vector.tensor_mul(xt[:, j, :], xt[:, j, :], bg_ap)
            if MUL_J3 == "gpsimd":
                nc.gpsimd.tensor_mul(xt[:, NJ - 1, :], xt[:, NJ - 1, :], bg_ap)
            else:
                nc.vector.tensor_mul(xt[:, NJ - 1, :], xt[:, NJ - 1, :], bg_ap)
            if OUT_ENGINE == "scalar":
                nc.scalar.dma_start(out=ys[b], in_=xt[:])
            else:
                nc.sync.dma_start(out=ys[b], in_=xt[:])
            if b + PREF < BLOC:
                dma_in(b + PREF)

        # software-pipelined emission: each engine's stream sees work in
        # data-readiness order, so in-order engines never head-of-line block
        for b in range(PREF):
            dma_in(b)
        phase1(0)
        phase2(0)
        for b in range(BLOC):
            if b + 1 < BLOC:
                phase1(b + 1)
            phase3(b)
            if b + 1 < BLOC:
                phase2(b + 1)
            phase4(b)


def _build_nc():
    nc = bacc.Bacc("TRN2", debug=False)
    xs = nc.dram_tensor("xs", [BLOC, NP, NJ, HW], F32, kind="ExternalInput")
    m8 = nc.dram_tensor("m8", [NP, NP], F32, kind="ExternalInput")
    wv = nc.dram_tensor("wv", [NP, 1], F32, kind="ExternalInput")
    bv = nc.dram_tensor("bv", [NP, 1], F32, kind="ExternalInput")
    ys = nc.dram_tensor("ys", [BLOC, NP, NJ, HW], F32, kind="ExternalOutput")
    with tile.TileContext(nc) as tc:
        _emit(tc, nc, xs, m8, wv, bv, ys)
    nc.compile()
    return nc


def get_nc():
    if "nc" not in _cache:
        _cache["nc"] = _build_nc()
    return _cache["nc"]


def make_in_maps(x, weight, bias):
    x = np.ascontiguousarray(np.asarray(x, dtype=np.float32))
    weight = np.asarray(weight, dtype=np.float32).reshape(G)
    bias = np.asarray(bias, dtype=np.float32).reshape(G)
    # [core, b, p, j, hw] with c = NJ*p + j
    xs = x.reshape(NCORES, BLOC, NP, NJ, HW)
    band = np.arange(NP) // PBAND
    m8 = (band[:, None] == band[None, :]).astype(np.float32)  # [NP, NP] indicator
    wv = np.ascontiguousarray(np.repeat(weight, PBAND)[:, None])
    bv = np.ascontiguousarray(np.repeat(bias, PBAND)[:, None])
    return [
        {"xs": np.ascontiguousarray(xs[i]), "m8": m8, "wv": wv, "bv": bv}
        for i in range(NCORES)
    ]


def run(x, weight, bias, trace=False, **spmd_kwargs):
    nc = get_nc()
    in_maps = make_in_maps(x, weight, bias)
    res = run_bass_kernel_spmd(
        nc, in_maps, core_ids=list(range(NCORES)), trace=trace, **spmd_kwargs
    )
    out = np.stack([res.results[i]["ys"] for i in range(NCORES)])
    return out.reshape(B, C, H, W), res


def kernel(x, weight, bias, groups=G, **_ignored):
    assert int(groups) == G
    out, _ = run(x, weight, bias, trace=False)
    return out
